# revision 1
# baseline (speedup 1.0000x reference)
"""Trainium2 Bass kernel for nn_CharacterMaskAttentionHead.

Sharding: 8 cores = 4 images x 2 H-halves (data-parallel over batch, spatial
split within each image). Bottom halves are H-flipped on the host so every core
runs the identical SPMD program on a "top slab" (true image edge at the top,
4-row interior halo at the bottom). GroupNorm statistics are exact: each core
reduces its half, per-pair partial sums are combined with a tiny (32x2 float)
in-kernel AllReduce that overlaps with kernel-branch compute.

Per core:
  - mask tower: 4x (3x3 conv -> GN(32) -> ReLU) on a padded [86 x 162] slab,
    conv as 9 shifted matmuls accumulating in PSUM (float32r, full PE rate).
  - kernel tower: replicated within each pair on the host-resized 40x40 grid
    (GN is local there).
  - top-k gather as PE transpose + one-hot selection matmul (host computes
    jax-exact top-k indices, baked into a per-core selection-matrix input).
  - kow 1x1 as 25 matmuls; dynamic-conv einsum as 4x27 matmuls -> DMA out.
Host: pads/flips/rounds inputs, applies the jax-exact bilinear-antialias
resize (0.7% of total FLOPs, memory-bound), computes top-k, reassembles.
"""
import sys
import contextlib

sys.path.insert(0, '/opt/trn_rl_repo')

import numpy as np

import concourse.bacc as bacc
import concourse.bass as bass
import concourse.tile as tile
from concourse import mybir
from concourse.bass_utils import run_bass_kernel_spmd
from concourse.masks import make_identity

F32 = mybir.dt.float32
F32R = mybir.dt.float32r

B, CIN, H, W = 4, 256, 160, 160
C, L, S, KSEL = 128, 25, 40, 16
CINP = CIN + 2
HALF = 80                 # output rows per core
SLAB = 84                 # conv rows per layer (80 + 4 halo)
ROWS, COLS = 86, 162      # padded slab buffer
GRID_P = S + 2            # padded 40x40 grid (42)
NT = 28                   # mask conv tiles (3 rows x 160)
KL = KSEL * L             # 400
NPOS = S * S              # 1600
NCH = 13                  # ceil(1600/128)
EPS = 1e-5

_PROGRAM_CACHE = {}


def _round_f32r(x):
    """Round fp32 to the f32r grid (12 mantissa bits) so DMA-fed float32r
    tiles hold values the PE will not re-round differently in sim vs HW."""
    b = np.ascontiguousarray(x, np.float32).view(np.uint32)
    b = ((b + 0x800) & np.uint32(0xFFFFF000)).astype(np.uint32)
    return b.view(np.float32)


def resize_weight_mat(in_size, out_size):
    """jax.image.resize bilinear+antialias weight matrix; out = Wmat @ in."""
    inv_scale = in_size / out_size
    kernel_scale = max(inv_scale, 1.0)
    sample_f = (np.arange(out_size) + 0.5) * inv_scale - 0.5
    x = np.abs(sample_f[:, None] - np.arange(in_size)[None, :]) / kernel_scale
    w = np.maximum(0.0, 1.0 - x)
    tot = w.sum(axis=1, keepdims=True)
    w = np.where(np.abs(tot) > 1000 * np.finfo(np.float32).eps, w / tot, 0.0)
    valid = (sample_f >= -0.5) & (sample_f <= in_size - 0.5)
    return np.where(valid[:, None], w, 0.0)


def topk_idx(vals, k):
    """jax.lax.top_k indices: descending value, ties -> lower index first."""
    return np.argsort(-vals, axis=-1, kind='stable')[..., :k]


# --------------------------------------------------------------------------
# device program
# --------------------------------------------------------------------------

def build_program(use_f32r=True, use_cc=True):
    # use_cc: True (all), False (none), or set of stat-stage indices 0..4
    cc_set = (set(range(5)) if use_cc is True
              else (set() if use_cc is False else set(use_cc)))
    key = (use_f32r, tuple(sorted(cc_set)))
    if key in _PROGRAM_CACHE:
        return _PROGRAM_CACHE[key]
    ADT = F32R if use_f32r else F32

    nc = bacc.Bacc("TRN2", target_bir_lowering=False, num_devices=8)

    img = nc.dram_tensor("img", (CINP, ROWS, COLS), ADT, kind="ExternalInput")
    kin = nc.dram_tensor("kin", (CINP, GRID_P, GRID_P), ADT, kind="ExternalInput")
    wdr = {}
    for pre in ("f", "k"):
        wdr[f"{pre}w0a"] = nc.dram_tensor(f"{pre}w0a", (128, 9, 128), ADT, kind="ExternalInput")
        wdr[f"{pre}w0b"] = nc.dram_tensor(f"{pre}w0b", (128, 9, 128), ADT, kind="ExternalInput")
        wdr[f"{pre}w0c"] = nc.dram_tensor(f"{pre}w0c", (2, 9, 128), ADT, kind="ExternalInput")
        for l in (1, 2, 3):
            wdr[f"{pre}w{l}"] = nc.dram_tensor(f"{pre}w{l}", (128, 9, 128), ADT, kind="ExternalInput")
    fowT_d = nc.dram_tensor("fowT", (128, 128), ADT, kind="ExternalInput")
    kowT_d = nc.dram_tensor("kowT", (128, C * L), F32, kind="ExternalInput")
    mask_gb_d = nc.dram_tensor("mask_gb", (128, 10), F32, kind="ExternalInput")
    k_gb_d = nc.dram_tensor("k_gb", (128, 8), F32, kind="ExternalInput")
    kobT_d = nc.dram_tensor("kobT", (128, L), F32, kind="ExternalInput")
    onehot_g_d = nc.dram_tensor("onehot_g", (128, 32), F32, kind="ExternalInput")
    onehot_b_d = nc.dram_tensor("onehot_b", (32, 128), F32, kind="ExternalInput")
    selP_d = nc.dram_tensor("selP", (128, NCH * KSEL), F32, kind="ExternalInput")
    out_d = nc.dram_tensor("out", (KL, HALF * W), F32, kind="ExternalOutput")
    cc_d = [(nc.dram_tensor(f"ccin{l}", (32, 2), F32),
             nc.dram_tensor(f"ccout{l}", (32, 2), F32)) for l in range(5)]

    with tile.TileContext(nc) as tc, contextlib.ExitStack() as ctx:
        consts = ctx.enter_context(tc.tile_pool(name="consts", bufs=1))
        acts = ctx.enter_context(tc.tile_pool(name="acts", bufs=1))
        wpool = ctx.enter_context(tc.tile_pool(name="wpool", bufs=3))
        small = ctx.enter_context(tc.tile_pool(name="small", bufs=1))
        stage = ctx.enter_context(tc.tile_pool(name="stage", bufs=2))
        kpool = ctx.enter_context(tc.tile_pool(name="kpool", bufs=1))
        ps_conv = ctx.enter_context(tc.tile_pool(name="ps_conv", bufs=4, space="PSUM"))
        ps_k = ctx.enter_context(tc.tile_pool(name="ps_k", bufs=2, space="PSUM"))
        ps_small = ctx.enter_context(tc.tile_pool(name="ps_small", bufs=2, space="PSUM"))

        # ------------- constants -------------
        def load_const(dram, shape, dt=F32):
            t = consts.tile(shape, dt, tag=dram.name, name=dram.name)
            nc.sync.dma_start(out=t, in_=dram[tuple(slice(None) for _ in shape)])
            return t

        onehot_g = load_const(onehot_g_d, [128, 32])
        onehot_b = load_const(onehot_b_d, [32, 128])
        mask_gb = load_const(mask_gb_d, [128, 10])
        k_gb = load_const(k_gb_d, [128, 8])
        kobT = load_const(kobT_d, [128, L])
        selP = load_const(selP_d, [128, NCH * KSEL])
        fowT = load_const(fowT_d, [128, 128], ADT)
        eps32 = consts.tile([32, 1], F32, tag="eps32", name="eps32")
        nc.vector.memset(eps32, EPS)
        ident = consts.tile([128, 128], F32, tag="ident", name="ident")
        make_identity(nc, ident)

        # activation slabs (ping-pong); pads zeroed via tensor_copy from an
        # F32 zero tile (memset cannot produce f32r, tensor_copy can)
        zsrc = consts.tile([128, COLS], F32, tag="zsrc", name="zsrc")
        nc.vector.memset(zsrc, 0.0)
        zcol = zsrc[:, 0:ROWS].rearrange("p (a b) -> p a b", b=1)

        X = [acts.tile([128, ROWS, COLS], ADT, tag="X0", name="X0"),
             acts.tile([128, ROWS, COLS], ADT, tag="X1", name="X1")]
        for x in X:
            nc.vector.tensor_copy(x[:, 0, :], zsrc[:, :])
            nc.vector.tensor_copy(x[:, ROWS - 1, :], zsrc[:, :])
            nc.vector.tensor_copy(x[:, :, 0:1], zcol)
            nc.vector.tensor_copy(x[:, :, COLS - 1:COLS], zcol)
        KB = [kpool.tile([128, GRID_P, GRID_P], ADT, tag="KB0", name="KB0"),
              kpool.tile([128, GRID_P, GRID_P], ADT, tag="KB1", name="KB1")]
        zcolk = zsrc[:, 0:GRID_P].rearrange("p (a b) -> p a b", b=1)
        for x in KB:
            nc.vector.tensor_copy(x[:, 0, :], zsrc[:, 0:GRID_P])
            nc.vector.tensor_copy(x[:, GRID_P - 1, :], zsrc[:, 0:GRID_P])
            nc.vector.tensor_copy(x[:, :, 0:1], zcolk)
            nc.vector.tensor_copy(x[:, :, GRID_P - 1:GRID_P], zcolk)

        # conv weights: shared-tag lazy rotation within each tower
        def load_w(nm, tag):
            dr = wdr[nm]
            t = wpool.tile([dr.shape[0], 9, 128], ADT, tag=tag, name=nm)
            nc.sync.dma_start(out=t, in_=dr[:, :, :])
            return t

        # kin chunks (tags shared with later k-branch tiles to save SBUF)
        kinA = kpool.tile([128, GRID_P, GRID_P], ADT, tag="sh_kow", name="sh_kow")
        kinB = kpool.tile([128, GRID_P, GRID_P], ADT, tag="sh_hidT", name="sh_hidT")
        kinC = kpool.tile([2, GRID_P, GRID_P], ADT, tag="sh_c", name="sh_c")
        nc.sync.dma_start(out=kinA, in_=kin[0:128, :, :])
        nc.sync.dma_start(out=kinB, in_=kin[128:256, :, :])
        nc.sync.dma_start(out=kinC, in_=kin[256:258, :, :])

        # ------------- GroupNorm helper -------------
        def emit_gn(buf, nrows_valid, ncols, divisor, gb_ap, cc, fam):
            """stats over buf rows [1, 1+nrows_valid), cols [1, 1+ncols);
            returns [128,2] (a|b) tile for the apply pass."""
            stats = small.tile([128, nrows_valid, 6], F32, tag=f"bnst{fam}", name=f"bnst{fam}")
            for r in range(nrows_valid):
                nc.vector.bn_stats(
                    out=stats[:, r, :],
                    in_=buf[:, 1 + r, 1:1 + ncols].bitcast(F32))
            mv = small.tile([128, 2], F32, tag=f"mv{fam}", name=f"mv{fam}")
            nc.vector.bn_aggr(out=mv[:], in_=stats.rearrange("p a b -> p (a b)"))

            sq = small.tile([128, 2], F32, tag=f"sq{fam}", name=f"sq{fam}")
            nc.vector.tensor_mul(sq[:, 1:2], mv[:, 0:1], mv[:, 0:1])
            nc.vector.tensor_add(sq[:, 1:2], sq[:, 1:2], mv[:, 1:2])
            nc.vector.tensor_copy(sq[:, 0:1], mv[:, 0:1])

            pg = ps_small.tile([32, 2], F32, tag="pss", name="pss")
            nc.tensor.matmul(pg, onehot_g[:], sq[:], start=True, stop=True)
            g32 = small.tile([32, 2], F32, tag=f"g32{fam}", name=f"g32{fam}")
            nc.vector.tensor_copy(g32[:], pg)

            if cc is not None:
                ccin_d, ccout_d = cc
                nc.sync.dma_start(out=ccin_d[:, :], in_=g32[:])
                nc.gpsimd.collective_compute(
                    "AllReduce", mybir.AluOpType.add,
                    replica_groups=[[0, 1], [2, 3], [4, 5], [6, 7]],
                    ins=[ccin_d[:, :]], outs=[ccout_d[:, :]])
                g32 = small.tile([32, 2], F32, tag=f"g32r{fam}", name=f"g32r{fam}")
                nc.sync.dma_start(out=g32[:], in_=ccout_d[:, :])

            nc.vector.tensor_scalar_mul(g32[:], g32[:], 1.0 / divisor)
            msq = small.tile([32, 1], F32, tag=f"msq{fam}", name=f"msq{fam}")
            nc.vector.tensor_mul(msq[:], g32[:, 0:1], g32[:, 0:1])
            nc.vector.tensor_sub(g32[:, 1:2], g32[:, 1:2], msq[:])
            nc.scalar.activation(out=g32[:, 1:2], in_=g32[:, 1:2],
                                 func=mybir.ActivationFunctionType.Sqrt,
                                 bias=eps32[:], scale=1.0)
            nc.vector.reciprocal(g32[:, 1:2], g32[:, 1:2])

            pb = ps_small.tile([128, 2], F32, tag="pss", name="pss")
            nc.tensor.matmul(pb, onehot_b[:], g32[:], start=True, stop=True)
            mr = small.tile([128, 2], F32, tag=f"mr{fam}", name=f"mr{fam}")
            nc.vector.tensor_copy(mr[:], pb)

            ab = small.tile([128, 2], F32, tag=f"ab{fam}", name=f"ab{fam}")
            nc.vector.tensor_mul(ab[:, 0:1], gb_ap[:, 0:1], mr[:, 1:2])
            nc.vector.tensor_mul(ab[:, 1:2], mr[:, 0:1], ab[:, 0:1])
            nc.vector.tensor_sub(ab[:, 1:2], gb_ap[:, 1:2], ab[:, 1:2])
            return ab

        def apply_gn(dst, ab, nrows, ncols, chunk=None):
            chunk = chunk or nrows
            r = 0
            while r < nrows:
                take = min(chunk, nrows - r)
                nc.scalar.activation(
                    out=dst[:, 1 + r:1 + r + take, 1:1 + ncols],
                    in_=dst[:, 1 + r:1 + r + take, 1:1 + ncols].bitcast(F32),
                    func=mybir.ActivationFunctionType.Relu,
                    scale=ab[:, 0:1], bias=ab[:, 1:2])
                r += take

        # ------------- conv emitters -------------
        def mask_conv_layer(src_fn, wts, dst):
            n_chunks = len(wts)
            for t in range(NT):
                ps = ps_conv.tile([128, 3, W], F32, tag="pc", name="pc")
                first = True
                for ci in range(n_chunks):
                    rhs_tile = src_fn(t, ci)
                    for tap in range(9):
                        dy, dx = divmod(tap, 3)
                        nc.tensor.matmul(
                            ps, wts[ci][:, tap, :],
                            rhs_tile[:, dy:dy + 3, dx:dx + W],
                            start=first,
                            stop=(ci == n_chunks - 1 and tap == 8))
                        first = False
                nc.vector.tensor_copy(dst[:, 1 + 3 * t:4 + 3 * t, 1:1 + W], ps)

        def k_conv_layer(srcs, wts, dst):
            n_chunks = len(wts)
            for t in range(4):
                ps = ps_k.tile([128, 10, S], F32, tag="pk", name="pk")
                first = True
                for ci in range(n_chunks):
                    for tap in range(9):
                        dy, dx = divmod(tap, 3)
                        nc.tensor.matmul(
                            ps, wts[ci][:, tap, :],
                            srcs[ci][:, 10 * t + dy:10 * t + dy + 10, dx:dx + S],
                            start=first,
                            stop=(ci == n_chunks - 1 and tap == 8))
                        first = False
                nc.vector.tensor_copy(dst[:, 1 + 10 * t:11 + 10 * t, 1:1 + S], ps)

        # ------------- mask layer 1 (streamed from DRAM) + k layer 1 -------------
        fw0 = [load_w("fw0a", "fw"), load_w("fw0b", "fw"), load_w("fw0c", "fw")]
        kw0 = [load_w("kw0a", "kw"), load_w("kw0b", "kw"), load_w("kw0c", "kw")]

        l1_stages = {}

        def l1_src(t, ci):
            if (t, ci) not in l1_stages:
                p = 2 if ci == 2 else 128
                st = stage.tile([p, 5, COLS], ADT, tag=f"st{ci}", name=f"st{ci}")
                nc.sync.dma_start(out=st, in_=img[128 * ci:128 * ci + p,
                                                  3 * t:3 * t + 5, :])
                l1_stages[(t, ci)] = st
            return l1_stages[(t, ci)]

        mask_conv_layer(l1_src, fw0, X[0])
        ab = emit_gn(X[0], HALF, W, 8.0, mask_gb[:, 0:2], cc_d[0] if 0 in cc_set else None, "m")
        k_conv_layer([kinA, kinB, kinC], kw0, KB[0])
        abk = emit_gn(KB[0], S, S, 4.0, k_gb[:, 0:2], None, "k")
        apply_gn(KB[0], abk, S, S)
        apply_gn(X[0], ab, SLAB, W, chunk=12)

        # ------------- layers 2..4 -------------
        cur = 0
        kcur = 0
        for l in (1, 2, 3):
            fwl = [load_w(f"fw{l}", "fw")]
            kwl = [load_w(f"kw{l}", "kw")]

            def src(t, ci, _cur=cur):
                return X[_cur][:, 3 * t:3 * t + 5, :]

            mask_conv_layer(src, fwl, X[1 - cur])
            ab = emit_gn(X[1 - cur], HALF, W, 8.0,
                         mask_gb[:, 2 * l:2 * l + 2], cc_d[l] if l in cc_set else None, "m")
            k_conv_layer([KB[kcur]], kwl, KB[1 - kcur])
            abk = emit_gn(KB[1 - kcur], S, S, 4.0, k_gb[:, 2 * l:2 * l + 2],
                          None, "k")
            apply_gn(KB[1 - kcur], abk, S, S)
            apply_gn(X[1 - cur], ab, SLAB, W, chunk=12)
            cur, kcur = 1 - cur, 1 - kcur

        khid = KB[kcur]   # l4 output (kcur flipped 3 times: KB[1])

        # ------------- fow + GN + relu -------------
        mf = X[1 - cur]
        for t in range(27):
            ps = ps_conv.tile([128, 3, W], F32, tag="pc", name="pc")
            nc.tensor.matmul(ps, fowT[:],
                             X[cur][:, 1 + 3 * t:4 + 3 * t, 1:1 + W],
                             start=True, stop=True)
            nc.vector.tensor_copy(mf[:, 1 + 3 * t:4 + 3 * t, 1:1 + W], ps)
        # ------------- gather + kow (fills fow's AllReduce bubble) -------------
        hidc = kpool.tile([128, S, S], F32, tag="KB0", name="KB0")  # reuse KB0's slot
        nc.vector.tensor_copy(hidc[:], khid[:, 1:1 + S, 1:1 + S].bitcast(F32))
        hidcf = hidc.rearrange("p a b -> p (a b)")
        hidT = kpool.tile([128, NCH, 128], F32, tag="sh_hidT", name="sh_hidT")
        for j in range(NCH):
            npos = min(128, NPOS - 128 * j)
            pt = ps_small.tile([128, 128], F32, tag="pss", name="pss")
            nc.tensor.transpose(pt[0:npos, :], hidcf[:, 128 * j:128 * j + npos],
                                ident)
            nc.vector.tensor_copy(hidT[0:npos, j, :], pt[0:npos, :])
        psel = ps_small.tile([128, KSEL], F32, tag="pss", name="pss")
        selPv = selP.rearrange("p (j k) -> p j k", j=NCH)
        for j in range(NCH):
            npos = min(128, NPOS - 128 * j)
            nc.tensor.matmul(psel, hidT[0:npos, j, :], selPv[0:npos, j, :],
                             start=(j == 0), stop=(j == NCH - 1))
        hsel = small.tile([128, KSEL], F32, tag="hsel", name="hsel")
        nc.vector.tensor_copy(hsel[:], psel)

        kowT = kpool.tile([128, C * L], F32, tag="sh_kow", name="sh_kow")
        nc.sync.dma_start(out=kowT, in_=kowT_d[:, :])
        kernT = small.tile([128, KL], F32, tag="kernT", name="kernT")
        kernTv = kernT.rearrange("p (k l) -> p k l", l=L)
        for l in range(L):
            pk = ps_small.tile([128, KSEL], F32, tag="pss", name="pss")
            nc.tensor.matmul(pk, kowT[:, 128 * l:128 * l + 128], hsel[:],
                             start=True, stop=True)
            nc.vector.tensor_scalar_add(kernTv[:, :, l], pk, kobT[:, l:l + 1])
        kernTr = small.tile([128, KL], ADT, tag="kernTr", name="kernTr")
        nc.vector.tensor_copy(kernTr[:], kernT[:])

        ab = emit_gn(mf, HALF, W, 8.0, mask_gb[:, 8:10], cc_d[4] if 4 in cc_set else None, "m")
        apply_gn(mf, ab, HALF + 1, W, chunk=12)

        # ------------- einsum + output -------------
        for (m0, mm) in ((0, 128), (128, 128), (256, 128), (384, KL - 384)):
            for t in range(27):
                rows = 3 if t < 26 else 2
                ps = ps_conv.tile([128, 3, W], F32, tag="pc", name="pc")
                nc.tensor.matmul(
                    ps[0:mm, 0:rows, :], kernTr[:, m0:m0 + mm],
                    mf[:, 1 + 3 * t:1 + 3 * t + rows, 1:1 + W],
                    start=True, stop=True)
                st = stage.tile([128, 3, W], F32, tag="st0", name="st0")
                if t % 2 == 0:
                    nc.vector.tensor_copy(st[0:mm, 0:rows, :], ps[0:mm, 0:rows, :])
                else:
                    nc.scalar.copy(out=st[0:mm, 0:rows, :], in_=ps[0:mm, 0:rows, :])
                nc.sync.dma_start(
                    out=out_d[m0:m0 + mm, 3 * W * t:3 * W * t + rows * W],
                    in_=st[0:mm, 0:rows, :])

    nc.compile()
    _PROGRAM_CACHE[key] = nc
    return nc


# --------------------------------------------------------------------------
# host glue
# --------------------------------------------------------------------------

def _prep_inputs(inputs, use_f32r=True):
    feats = np.asarray(inputs['feats'], np.float32)
    bboxes = np.asarray(inputs['matched_bboxes'])
    idx = topk_idx(bboxes, KSEL)

    rnd = _round_f32r if use_f32r else (lambda x: np.ascontiguousarray(x, np.float32))

    xx, yy = np.meshgrid(np.linspace(-1, 1, W, dtype=np.float64),
                         np.linspace(-1, 1, H, dtype=np.float64), indexing='xy')
    coord = np.stack([xx, yy]).astype(np.float32)
    Rh = resize_weight_mat(H, S).astype(np.float32)
    Rw = resize_weight_mat(W, S).astype(np.float32)

    def conv_w_prep(w, flip):
        w = np.asarray(w, np.float32)
        if flip:
            w = w[:, :, ::-1, :]
        return np.ascontiguousarray(
            w.transpose(1, 2, 3, 0).reshape(w.shape[1], 9, w.shape[0]))

    def gb_pack(pairs):
        out = np.zeros((128, 2 * len(pairs)), np.float32)
        for i, (g, b) in enumerate(pairs):
            out[:, 2 * i] = np.asarray(g, np.float32)
            out[:, 2 * i + 1] = np.asarray(b, np.float32)
        return out

    mask_gb = gb_pack([(inputs[f'fg{l}'], inputs[f'fb{l}']) for l in range(4)]
                      + [(inputs['fog'], inputs['fob'])])
    k_gb = gb_pack([(inputs[f'kg{l}'], inputs[f'kb{l}']) for l in range(4)])

    kow = np.asarray(inputs['kow'], np.float32).reshape(C * L, C)
    kowT = np.ascontiguousarray(kow.T)
    kob = np.asarray(inputs['kob'], np.float32)
    kobT = np.ascontiguousarray(kob.reshape(L, C).T)
    fow = np.asarray(inputs['fow'], np.float32).reshape(C, C)
    fowT = rnd(np.ascontiguousarray(fow.T))

    onehot_g = np.zeros((128, 32), np.float32)
    onehot_g[np.arange(128), np.arange(128) // 4] = 1.0
    onehot_b = np.ascontiguousarray(onehot_g.T)

    per_orient = {}
    for flip in (False, True):
        wd = {}
        for pre, key in (("f", "fw"), ("k", "kw")):
            w0 = conv_w_prep(inputs[f'{key}0'], flip)
            wd[f"{pre}w0a"] = rnd(w0[0:128])
            wd[f"{pre}w0b"] = rnd(w0[128:256])
            wd[f"{pre}w0c"] = rnd(w0[256:258])
            for l in (1, 2, 3):
                wd[f"{pre}w{l}"] = rnd(conv_w_prep(inputs[f'{key}{l}'], flip))
        per_orient[flip] = wd

    in_maps = []
    for c in range(8):
        b, s = c // 2, c % 2
        base = np.concatenate([feats[b], coord], 0)
        if s == 1:
            base = base[:, ::-1, :]
        img_pad = np.zeros((CINP, ROWS, COLS), np.float32)
        img_pad[:, 1:, 1:1 + W] = base[:, 0:ROWS - 1, :]

        t = np.tensordot(Rh, base, axes=(1, 1))
        kin = np.tensordot(t, Rw, axes=(2, 1)).transpose(1, 0, 2)
        kin_pad = np.zeros((CINP, GRID_P, GRID_P), np.float32)
        kin_pad[:, 1:-1, 1:-1] = kin

        selP = np.zeros((128, NCH * KSEL), np.float32)
        for k in range(KSEL):
            y_, x_ = divmod(int(idx[b, k]), S)
            if s == 1:
                y_ = S - 1 - y_
            p = y_ * S + x_
            selP[p % 128, (p // 128) * KSEL + k] = 1.0

        m = dict(per_orient[s == 1])
        m.update(
            img=rnd(img_pad), kin=rnd(kin_pad),
            fowT=fowT, kowT=kowT, mask_gb=mask_gb, k_gb=k_gb, kobT=kobT,
            onehot_g=onehot_g, onehot_b=onehot_b, selP=selP)
        in_maps.append(m)
    return in_maps


def assemble_output(results):
    out = np.zeros((B, KL, H, W), np.float32)
    for c in range(8):
        b, s = c // 2, c % 2
        pred = results[c]["out"].reshape(KL, HALF, W)
        if s == 0:
            out[b, :, 0:HALF, :] = pred
        else:
            out[b, :, HALF:, :] = pred[:, ::-1, :]
    return out.reshape(B, KSEL, L, H, W)


def kernel(**inputs) -> np.ndarray:
    use_f32r = True
    nc = build_program(use_f32r=use_f32r)
    in_maps = _prep_inputs(inputs, use_f32r=use_f32r)
    res = run_bass_kernel_spmd(nc, in_maps, core_ids=list(range(8)))
    return assemble_output(res.results)



# revision 20
# speedup vs baseline: 1.2840x; 1.2840x over previous
"""Trainium2 Bass kernel for nn_CharacterMaskAttentionHead.

Sharding: 8 cores = 4 images x 2 H-halves (data-parallel over batch, spatial
split within each image). Bottom halves are H-flipped on the host so every core
runs the identical SPMD program on a "top slab" (true image edge at the top,
4-row interior halo at the bottom). GroupNorm statistics are exact: each core
reduces its half, per-pair partial sums are combined with a tiny (32x2 float)
in-kernel AllReduce.

Performance structure (v2):
  - The coord-channel contribution of both layer-1 convs is precomputed on the
    host (coords are fixed ramps) and added on VectorE during the PSUM->SBUF
    copy, removing the 9 extra 2-partition matmuls per tile.
  - bn_stats is batched per conv tile and interleaved with the conv copies, so
    group stats are ready right after the last conv matmul of a layer.
  - Each mask-layer GroupNorm is split into head (reduce + start AllReduce) and
    tail (finish stats -> scale/bias -> apply); the kernel-branch conv of the
    same layer is emitted between head and tail so the PE stays busy during the
    ~11us collective round trip (keeps HAM at full clock).
  - The final fow GN bubble is filled with the top-k gather (PE transpose +
    one-hot matmul) and the kow 1x1 (single weight load, 7 N=512 matmuls).
  - The dynamic-conv einsum output is cast to bf16 on-chip and written with
    grouped DMAs alternating between the Sync and GpSimd queues.
Host: pads/flips/rounds inputs, applies the jax-exact bilinear-antialias
resize, computes top-k, computes the coord maps, reassembles.
"""
import sys
import contextlib

sys.path.insert(0, '/opt/trn_rl_repo')

import numpy as np

import concourse.bacc as bacc
import concourse.bass as bass
import concourse.tile as tile
from concourse import mybir
from concourse.bass_utils import run_bass_kernel_spmd
from concourse.masks import make_identity

F32 = mybir.dt.float32
F32R = mybir.dt.float32r
BF16 = mybir.dt.bfloat16

B, CIN, H, W = 4, 256, 160, 160
C, L, S, KSEL = 128, 25, 40, 16
HALF = 80                 # output rows per core
SLAB = 84                 # conv rows for layer 1 (80 + 4 halo)
ROWS, COLS = 86, 162      # padded slab buffer
GRID_P = S + 2            # padded 40x40 grid (42)
NT = 28                   # mask conv tiles (3 rows x 160)
KL = KSEL * L             # 400
NPOS = S * S              # 1600
NCH = 13                  # ceil(1600/128)
EPS = 1e-5

_PROGRAM_CACHE = {}


def _round_f32r(x):
    """Round fp32 to the f32r grid (12 mantissa bits) so DMA-fed float32r
    tiles hold values the PE will not re-round differently in sim vs HW."""
    b = np.ascontiguousarray(x, np.float32).view(np.uint32)
    b = ((b + 0x800) & np.uint32(0xFFFFF000)).astype(np.uint32)
    return b.view(np.float32)


def resize_weight_mat(in_size, out_size):
    """jax.image.resize bilinear+antialias weight matrix; out = Wmat @ in."""
    inv_scale = in_size / out_size
    kernel_scale = max(inv_scale, 1.0)
    sample_f = (np.arange(out_size) + 0.5) * inv_scale - 0.5
    x = np.abs(sample_f[:, None] - np.arange(in_size)[None, :]) / kernel_scale
    w = np.maximum(0.0, 1.0 - x)
    tot = w.sum(axis=1, keepdims=True)
    w = np.where(np.abs(tot) > 1000 * np.finfo(np.float32).eps, w / tot, 0.0)
    valid = (sample_f >= -0.5) & (sample_f <= in_size - 0.5)
    return np.where(valid[:, None], w, 0.0)


def topk_idx(vals, k):
    """jax.lax.top_k indices: descending value, ties -> lower index first."""
    return np.argsort(-vals, axis=-1, kind='stable')[..., :k]


def coord_conv_map(w0c, cimg, nrows):
    """'SAME' 3x3 conv of the 2-channel coord image with the coord slice of
    the first conv weight; returns [128, nrows, W'] fp32."""
    ch, hh, ww = cimg.shape
    cp = np.zeros((ch, hh + 2, ww + 2), np.float32)
    cp[:, 1:-1, 1:-1] = cimg
    out = np.zeros((w0c.shape[0], hh, ww), np.float32)
    for c in range(ch):
        for dy in range(3):
            for dx in range(3):
                out += (w0c[:, c, dy, dx][:, None, None]
                        * cp[c, dy:dy + hh, dx:dx + ww])
    return np.ascontiguousarray(out[:, 0:nrows])


# --------------------------------------------------------------------------
# device program
# --------------------------------------------------------------------------

def build_program(use_f32r=True, use_cc=True):
    import os as _os
    out_bf16 = _os.environ.get('OUT_BF16', '1') == '1'
    stats_ttr = _os.environ.get('STATS', 'ttr') == 'ttr'
    cc_set = (set(range(5)) if use_cc is True
              else (set() if use_cc is False else set(use_cc)))
    key = (use_f32r, tuple(sorted(cc_set)), out_bf16, stats_ttr)
    if key in _PROGRAM_CACHE:
        return _PROGRAM_CACHE[key]
    ADT = F32R if use_f32r else F32

    nc = bacc.Bacc("TRN2", target_bir_lowering=False, num_devices=8)

    img = nc.dram_tensor("img", (CIN, ROWS, COLS), ADT, kind="ExternalInput")
    kin = nc.dram_tensor("kin", (CIN, GRID_P, GRID_P), ADT, kind="ExternalInput")
    cmapm_d = nc.dram_tensor("cmapm", (128, SLAB, W), F32, kind="ExternalInput")
    cmapk_d = nc.dram_tensor("cmapk", (128, S, S), F32, kind="ExternalInput")
    wdr = {}
    for pre in ("f", "k"):
        wdr[f"{pre}w0a"] = nc.dram_tensor(f"{pre}w0a", (128, 9, 128), ADT, kind="ExternalInput")
        wdr[f"{pre}w0b"] = nc.dram_tensor(f"{pre}w0b", (128, 9, 128), ADT, kind="ExternalInput")
        for l in (1, 2, 3):
            wdr[f"{pre}w{l}"] = nc.dram_tensor(f"{pre}w{l}", (128, 9, 128), ADT, kind="ExternalInput")
    fowT_d = nc.dram_tensor("fowT", (128, 128), ADT, kind="ExternalInput")
    kowT_d = nc.dram_tensor("kowT", (128, C * L), ADT, kind="ExternalInput")
    mask_gb_d = nc.dram_tensor("mask_gb", (128, 10), F32, kind="ExternalInput")
    k_gb_d = nc.dram_tensor("k_gb", (128, 8), F32, kind="ExternalInput")
    kobT_d = nc.dram_tensor("kobT", (128, L), F32, kind="ExternalInput")
    onehot_g_d = nc.dram_tensor("onehot_g", (128, 32), F32, kind="ExternalInput")
    onehot_b_d = nc.dram_tensor("onehot_b", (32, 128), F32, kind="ExternalInput")
    selP_d = nc.dram_tensor("selP", (128, NCH * KSEL), F32, kind="ExternalInput")
    ODT = BF16 if out_bf16 else F32
    out_d = nc.dram_tensor("out", (KL, HALF * W), ODT, kind="ExternalOutput")
    cc_d = [(nc.dram_tensor(f"ccin{l}", (32, 2), F32),
             nc.dram_tensor(f"ccout{l}", (32, 2), F32)) for l in range(6)]

    with tile.TileContext(nc) as tc, contextlib.ExitStack() as ctx:
        consts = ctx.enter_context(tc.tile_pool(name="consts", bufs=1))
        acts = ctx.enter_context(tc.tile_pool(name="acts", bufs=1))
        wpool = ctx.enter_context(tc.tile_pool(name="wpool", bufs=2))
        small = ctx.enter_context(tc.tile_pool(name="small", bufs=1))
        stage = ctx.enter_context(tc.tile_pool(name="stage", bufs=2))
        kpool = ctx.enter_context(tc.tile_pool(name="kpool", bufs=1))
        ps_conv = ctx.enter_context(tc.tile_pool(name="ps_conv", bufs=4, space="PSUM"))
        ps_k = ctx.enter_context(tc.tile_pool(name="ps_k", bufs=2, space="PSUM"))
        ps_small = ctx.enter_context(tc.tile_pool(name="ps_small", bufs=2, space="PSUM"))

        import os
        _dma = {'sync': nc.sync, 'scalar': nc.scalar, 'gpsimd': nc.gpsimd}[os.environ.get('DMA_ENG', 'mixed')] if os.environ.get('DMA_ENG', 'mixed') != 'mixed' else None
        _sc = _dma or nc.scalar
        _gp = _dma or nc.gpsimd
        # ------------- constants -------------
        def load_const(dram, shape, dt=F32, eng=None):
            t = consts.tile(shape, dt, tag=dram.name, name=dram.name)
            (eng or nc.sync).dma_start(out=t, in_=dram[tuple(slice(None) for _ in shape)])
            return t

        onehot_g = load_const(onehot_g_d, [128, 32])
        onehot_b = load_const(onehot_b_d, [32, 128])
        mask_gb = load_const(mask_gb_d, [128, 10])
        k_gb = load_const(k_gb_d, [128, 8])
        kobT = load_const(kobT_d, [128, L])
        selP = load_const(selP_d, [128, NCH * KSEL])
        fowT = load_const(fowT_d, [128, 128], ADT)
        cmapk = load_const(cmapk_d, [128, S, S], F32, eng=None)
        eps32 = consts.tile([32, 1], F32, tag="eps32", name="eps32")
        nc.vector.memset(eps32, EPS)
        ident = consts.tile([128, 128], F32, tag="ident", name="ident")
        make_identity(nc, ident)

        # activation slabs (ping-pong); pads zeroed via tensor_copy from an
        # F32 zero tile (memset cannot produce f32r, tensor_copy can)
        zsrc = consts.tile([128, COLS], F32, tag="zsrc", name="zsrc")
        nc.vector.memset(zsrc, 0.0)
        zcol = zsrc[:, 0:ROWS].rearrange("p (a b) -> p a b", b=1)

        X = [acts.tile([128, ROWS, COLS], ADT, tag="X0", name="X0"),
             acts.tile([128, ROWS, COLS], ADT, tag="X1", name="X1")]
        for x in X:
            nc.vector.tensor_copy(x[:, 0, :], zsrc[:, :])
            nc.vector.tensor_copy(x[:, ROWS - 1, :], zsrc[:, :])
            nc.vector.tensor_copy(x[:, :, 0:1], zcol)
            nc.vector.tensor_copy(x[:, :, COLS - 1:COLS], zcol)
        KB = [kpool.tile([128, GRID_P, GRID_P], ADT, tag="KB0", name="KB0"),
              kpool.tile([128, GRID_P, GRID_P], ADT, tag="KB1", name="KB1")]
        zcolk = zsrc[:, 0:GRID_P].rearrange("p (a b) -> p a b", b=1)
        for x in KB:
            nc.vector.tensor_copy(x[:, 0, :], zsrc[:, 0:GRID_P])
            nc.vector.tensor_copy(x[:, GRID_P - 1, :], zsrc[:, 0:GRID_P])
            nc.vector.tensor_copy(x[:, :, 0:1], zcolk)
            nc.vector.tensor_copy(x[:, :, GRID_P - 1:GRID_P], zcolk)

        # conv weights: shared-tag lazy rotation within each tower
        def load_w(nm, tag):
            dr = wdr[nm]
            t = wpool.tile([dr.shape[0], 9, 128], ADT, tag=tag, name=nm)
            nc.sync.dma_start(out=t, in_=dr[:, :, :])
            return t

        # kin chunks (tags shared with later k-branch tiles to save SBUF)
        kinA = kpool.tile([128, GRID_P, GRID_P], ADT, tag="sh_kow", name="sh_kow")
        kinB = kpool.tile([128, GRID_P, GRID_P], ADT, tag="sh_hidT", name="sh_hidT")
        _gp.dma_start(out=kinA, in_=kin[0:128, :, :])
        _gp.dma_start(out=kinB, in_=kin[128:256, :, :])

        # warm the collectives firmware path so the first real AllReduce
        # does not pay the ~11us first-call start delay (runs under L1 conv)
        if cc_set and os.environ.get('NO_WARM_CC', '') != '1':
            _gp.dma_start(out=cc_d[5][0][:, :], in_=zsrc[0:32, 0:2])
            nc.gpsimd.collective_compute(
                "AllReduce", mybir.AluOpType.add,
                replica_groups=[[0, 1], [2, 3], [4, 5], [6, 7]],
                ins=[cc_d[5][0][:, :]], outs=[cc_d[5][1][:, :]])

        # ------------- GroupNorm helpers -------------
        # per-conv-tile partial sums / sums-of-squares (fused into the
        # PSUM->SBUF copy via tensor_tensor_reduce)
        zeros3 = consts.tile([128, 3, W], F32, tag="zeros3", name="zeros3")
        nc.vector.memset(zeros3, 0.0)
        sqd = small.tile([128, 3, W], F32, tag="sqd", name="sqd")
        ksqd = small.tile([128, 10, S], F32, tag="ksqd", name="ksqd")
        kzeros = consts.tile([128, 10, S], F32, tag="kzeros", name="kzeros")
        nc.vector.memset(kzeros, 0.0)
        statsm = small.tile([128, HALF, 6], F32, tag="bnstm", name="bnstm")
        statsk = small.tile([128, S, 6], F32, tag="bnstk", name="bnstk")
        msum = small.tile([128, NT], F32, tag="msum", name="msum")
        msqs = small.tile([128, NT], F32, tag="msqs", name="msqs")
        ksum = small.tile([128, 4], F32, tag="ksum", name="ksum")
        ksqs = small.tile([128, 4], F32, tag="ksqs", name="ksqs")

        def gn_head(sums, sqs, nvalid, divisor, cc, fam):
            """reduce tile partials -> group-reduce -> (start AllReduce).
            Returns the local [32,2] (mean|E[x^2]) tile, pre-divided."""
            sq = small.tile([128, 2], F32, tag=f"sq{fam}", name=f"sq{fam}")
            if stats_ttr:
                nc.vector.tensor_reduce(out=sq[:, 0:1], in_=sums[:, 0:nvalid],
                                        axis=mybir.AxisListType.X,
                                        op=mybir.AluOpType.add)
                nc.vector.tensor_reduce(out=sq[:, 1:2], in_=sqs[:, 0:nvalid],
                                        axis=mybir.AxisListType.X,
                                        op=mybir.AluOpType.add)
            else:
                stats, nrows, npx = sums
                mv = small.tile([128, 2], F32, tag=f"mv{fam}", name=f"mv{fam}")
                nc.vector.bn_aggr(out=mv[:], in_=stats[:, 0:nrows, :]
                                  .rearrange("p a b -> p (a b)"))
                nc.vector.tensor_mul(sq[:, 1:2], mv[:, 0:1], mv[:, 0:1])
                nc.vector.tensor_add(sq[:, 1:2], sq[:, 1:2], mv[:, 1:2])
                nc.vector.tensor_copy(sq[:, 0:1], mv[:, 0:1])
                nc.vector.tensor_scalar_mul(sq[:], sq[:], float(npx))
            pg = ps_small.tile([32, 2], F32, tag="pss", name="pss")
            nc.tensor.matmul(pg, onehot_g[:], sq[:], start=True, stop=True)
            g32 = small.tile([32, 2], F32, tag=f"g32{fam}", name=f"g32{fam}")
            nc.vector.tensor_scalar_mul(g32[:], pg, 1.0 / divisor)
            if cc is not None:
                ccin_d, ccout_d = cc
                _sc.dma_start(out=ccin_d[:, :], in_=g32[:])
                nc.gpsimd.collective_compute(
                    "AllReduce", mybir.AluOpType.add,
                    replica_groups=[[0, 1], [2, 3], [4, 5], [6, 7]],
                    ins=[ccin_d[:, :]], outs=[ccout_d[:, :]])
            return g32

        def gn_tail(g32, cc, gb_ap, fam):
            """finish group stats -> per-channel scale(a)|bias(b) [128,2]."""
            if cc is not None:
                g32 = small.tile([32, 2], F32, tag=f"g32r{fam}", name=f"g32r{fam}")
                _sc.dma_start(out=g32[:], in_=cc[1][:, :])
            msq = small.tile([32, 1], F32, tag=f"msq{fam}", name=f"msq{fam}")
            nc.vector.tensor_mul(msq[:], g32[:, 0:1], g32[:, 0:1])
            nc.vector.tensor_sub(g32[:, 1:2], g32[:, 1:2], msq[:])
            nc.scalar.activation(out=g32[:, 1:2], in_=g32[:, 1:2],
                                 func=mybir.ActivationFunctionType.Sqrt,
                                 bias=eps32[:], scale=1.0)
            nc.vector.reciprocal(g32[:, 1:2], g32[:, 1:2])
            pb = ps_small.tile([128, 2], F32, tag="pss", name="pss")
            nc.tensor.matmul(pb, onehot_b[:], g32[:], start=True, stop=True)
            mr = small.tile([128, 2], F32, tag=f"mr{fam}", name=f"mr{fam}")
            nc.vector.tensor_copy(mr[:], pb)
            ab = small.tile([128, 2], F32, tag=f"ab{fam}", name=f"ab{fam}")
            nc.vector.tensor_mul(ab[:, 0:1], gb_ap[:, 0:1], mr[:, 1:2])
            nc.vector.tensor_mul(ab[:, 1:2], mr[:, 0:1], ab[:, 0:1])
            nc.vector.tensor_sub(ab[:, 1:2], gb_ap[:, 1:2], ab[:, 1:2])
            return ab

        def apply_gn(dst, ab, nrows, ncols, first=6, chunk=13):
            r = 0
            while r < nrows:
                take = min(first if r == 0 else chunk, nrows - r)
                nc.scalar.activation(
                    out=dst[:, 1 + r:1 + r + take, 1:1 + ncols],
                    in_=dst[:, 1 + r:1 + r + take, 1:1 + ncols].bitcast(F32),
                    func=mybir.ActivationFunctionType.Relu,
                    scale=ab[:, 0:1], bias=ab[:, 1:2])
                r += take

        # ------------- conv emitters (stats fused into the copy) -------------
        def copy_stats(dst_ap, ps, in1, rows_v, rows, slot_s, slot_q):
            """dst = ps + in1 (copy), accumulating sum / sum-of-squares over
            the first rows_v rows into the given partial-stat slots."""
            if rows_v == 0:
                nc.vector.tensor_add(dst_ap, ps, in1)
                return
            nc.vector.tensor_tensor_reduce(
                out=dst_ap[:, 0:rows_v, :], in0=ps[:, 0:rows_v, :],
                in1=in1[:, 0:rows_v, :], scale=1.0, scalar=0.0,
                op0=mybir.AluOpType.add, op1=mybir.AluOpType.add,
                accum_out=slot_s)
            if rows_v < rows:
                nc.vector.tensor_add(dst_ap[:, rows_v:rows, :],
                                     ps[:, rows_v:rows, :],
                                     in1[:, rows_v:rows, :])
            d32 = dst_ap[:, 0:rows_v, :].bitcast(F32)
            nc.vector.tensor_tensor_reduce(
                out=sqd[:, 0:rows_v, 0:dst_ap.shape[2]], in0=d32, in1=d32,
                scale=1.0, scalar=0.0,
                op0=mybir.AluOpType.mult, op1=mybir.AluOpType.add,
                accum_out=slot_q)

        def copy_stats_bn(dst_ap, ps, in1, rows_v, rows, stats, srow):
            nc.vector.tensor_add(dst_ap, ps, in1)
            for r in range(rows_v):
                nc.vector.bn_stats(out=stats[:, srow + r, :],
                                   in_=dst_ap[:, r, :].bitcast(F32))

        def mask_conv_layer(src_fn, wts, dst, ntiles=NT, cmap_fn=None):
            n_chunks = len(wts)
            for t in range(ntiles):
                ps = ps_conv.tile([128, 3, W], F32, tag="pc", name="pc")
                first = True
                for ci in range(n_chunks):
                    rhs_tile = src_fn(t, ci)
                    for tap in range(9):
                        dy, dx = divmod(tap, 3)
                        nc.tensor.matmul(
                            ps, wts[ci][:, tap, :],
                            rhs_tile[:, dy:dy + 3, dx:dx + W],
                            start=first,
                            stop=(ci == n_chunks - 1 and tap == 8))
                        first = False
                in1 = cmap_fn(t) if cmap_fn is not None else zeros3
                rows_v = 3 if t <= 25 else (2 if t == 26 else 0)
                if stats_ttr:
                    copy_stats(dst[:, 1 + 3 * t:4 + 3 * t, 1:1 + W], ps, in1,
                               rows_v, 3, msum[:, t:t + 1], msqs[:, t:t + 1])
                else:
                    copy_stats_bn(dst[:, 1 + 3 * t:4 + 3 * t, 1:1 + W], ps,
                                  in1, rows_v, 3, statsm, 3 * t)

        def k_conv_layer(srcs, wts, dst, cmap=None):
            n_chunks = len(wts)
            for t in range(4):
                ps = ps_k.tile([128, 10, S], F32, tag="pk", name="pk")
                first = True
                for ci in range(n_chunks):
                    for tap in range(9):
                        dy, dx = divmod(tap, 3)
                        nc.tensor.matmul(
                            ps, wts[ci][:, tap, :],
                            srcs[ci][:, 10 * t + dy:10 * t + dy + 10, dx:dx + S],
                            start=first,
                            stop=(ci == n_chunks - 1 and tap == 8))
                        first = False
                dst_ap = dst[:, 1 + 10 * t:11 + 10 * t, 1:1 + S]
                kin1 = (cmap[:, 10 * t:10 * t + 10, :] if cmap is not None
                        else kzeros)
                if stats_ttr:
                    nc.vector.tensor_tensor_reduce(
                        out=dst_ap, in0=ps, in1=kin1, scale=1.0, scalar=0.0,
                        op0=mybir.AluOpType.add, op1=mybir.AluOpType.add,
                        accum_out=ksum[:, t:t + 1])
                    d32 = dst_ap.bitcast(F32)
                    nc.vector.tensor_tensor_reduce(
                        out=ksqd[:], in0=d32, in1=d32, scale=1.0, scalar=0.0,
                        op0=mybir.AluOpType.mult, op1=mybir.AluOpType.add,
                        accum_out=ksqs[:, t:t + 1])
                else:
                    copy_stats_bn(dst_ap, ps, kin1, 10, 10, statsk, 10 * t)

        def k_gn_apply(buf, l):
            g32k = gn_head(ksum if stats_ttr else (statsk, S, S * S),
                           ksqs, 4, 4.0 * S * S, None, "k")
            abk = gn_tail(g32k, None, k_gb[:, 2 * l:2 * l + 2], "k")
            apply_gn(buf, abk, S, S, first=20, chunk=20)

        # ------------- mask layer 1 (streamed from DRAM) + k layer 1 -------------
        fw0 = [load_w("fw0a", "fw"), load_w("fw0b", "fw")]
        kw0 = [load_w("kw0a", "kw"), load_w("kw0b", "kw")]

        l1_stages = {}

        def l1_src(t, ci):
            if (t, ci) not in l1_stages:
                st = stage.tile([128, 5, COLS], ADT, tag=f"st{ci}", name=f"st{ci}")
                nc.sync.dma_start(out=st, in_=img[128 * ci:128 * ci + 128,
                                                  3 * t:3 * t + 5, :])
                l1_stages[(t, ci)] = st
            return l1_stages[(t, ci)]

        cm_stages = {}

        def l1_cmap(t):
            if t not in cm_stages:
                cm = stage.tile([128, 3, W], F32, tag="cm", name="cm")
                _sc.dma_start(out=cm, in_=cmapm_d[:, 3 * t:3 * t + 3, :])
                cm_stages[t] = cm
            return cm_stages[t]

        mask_conv_layer(l1_src, fw0, X[0], cmap_fn=l1_cmap)
        g32 = gn_head(msum if stats_ttr else (statsm, HALF, HALF * W),
                      msqs, 27, 8.0 * HALF * W,
                      cc_d[0] if 0 in cc_set else None, "m")
        # ---- bubble filler: k-tower layer 1 ----
        k_conv_layer([kinA, kinB], kw0, KB[0], cmap=cmapk)
        k_gn_apply(KB[0], 0)
        ab = gn_tail(g32, cc_d[0] if 0 in cc_set else None, mask_gb[:, 0:2], "m")
        apply_gn(X[0], ab, SLAB, W)

        # ------------- layers 2..4 -------------
        cur = 0
        kcur = 0
        for l in (1, 2, 3):
            fwl = [load_w(f"fw{l}", "fw")]
            kwl = [load_w(f"kw{l}", "kw")]

            def src(t, ci, _cur=cur):
                return X[_cur][:, 3 * t:3 * t + 5, :]

            ntiles = 27 if l == 3 else NT
            mask_conv_layer(src, fwl, X[1 - cur], ntiles=ntiles)
            g32 = gn_head(msum if stats_ttr else (statsm, HALF, HALF * W),
                          msqs, 27, 8.0 * HALF * W,
                          cc_d[l] if l in cc_set else None, "m")
            # ---- bubble filler: k-tower layer l ----
            k_conv_layer([KB[kcur]], kwl, KB[1 - kcur])
            if l == 1:
                # kow weights: load into kinA's slot (dead after k layer 1)
                kowT = kpool.tile([128, C * L], ADT, tag="sh_kow", name="kowT")
                nc.sync.dma_start(out=kowT, in_=kowT_d[:, :])
            k_gn_apply(KB[1 - kcur], l)
            ab = gn_tail(g32, cc_d[l] if l in cc_set else None,
                         mask_gb[:, 2 * l:2 * l + 2], "m")
            apply_gn(X[1 - cur], ab, SLAB if l < 3 else 81, W)
            cur, kcur = 1 - cur, 1 - kcur

        khid = KB[kcur]   # l4 output (kcur flipped 3 times: KB[1])

        # ------------- fow + GN (+ gather/kow in the collective bubble) -----
        mf = X[1 - cur]
        for t in range(27):
            ps = ps_conv.tile([128, 3, W], F32, tag="pc", name="pc")
            nc.tensor.matmul(ps, fowT[:],
                             X[cur][:, 1 + 3 * t:4 + 3 * t, 1:1 + W],
                             start=True, stop=True)
            rows_v = 3 if t <= 25 else 2
            if stats_ttr:
                copy_stats(mf[:, 1 + 3 * t:4 + 3 * t, 1:1 + W], ps, zeros3,
                           rows_v, 3, msum[:, t:t + 1], msqs[:, t:t + 1])
            else:
                copy_stats_bn(mf[:, 1 + 3 * t:4 + 3 * t, 1:1 + W], ps,
                              zeros3, rows_v, 3, statsm, 3 * t)
        g32 = gn_head(msum if stats_ttr else (statsm, HALF, HALF * W),
                      msqs, 27, 8.0 * HALF * W,
                      cc_d[4] if 4 in cc_set else None, "m")

        # ---- bubble filler: gather + kow ----
        hidc = kpool.tile([128, S, S], F32, tag="KB0", name="KB0")  # reuse KB0's slot
        nc.vector.tensor_copy(hidc[:], khid[:, 1:1 + S, 1:1 + S].bitcast(F32))
        hidcf = hidc.rearrange("p a b -> p (a b)")
        hidT = kpool.tile([128, NCH, 128], F32, tag="sh_hidT", name="hidT")
        for j in range(NCH):
            npos = min(128, NPOS - 128 * j)
            pt = ps_small.tile([128, 128], F32, tag="pss", name="pss")
            nc.tensor.transpose(pt[0:npos, :], hidcf[:, 128 * j:128 * j + npos],
                                ident)
            nc.vector.tensor_copy(hidT[0:npos, j, :], pt[0:npos, :])
        psel = ps_small.tile([128, KSEL], F32, tag="pss", name="pss")
        selPv = selP.rearrange("p (j k) -> p j k", j=NCH)
        for j in range(NCH):
            npos = min(128, NPOS - 128 * j)
            nc.tensor.matmul(psel, hidT[0:npos, j, :], selPv[0:npos, j, :],
                             start=(j == 0), stop=(j == NCH - 1))
        hsel = small.tile([128, KSEL], ADT, tag="hsel", name="hsel")
        nc.vector.tensor_copy(hsel[:], psel)

        # kow: kern^T[(k), (l,c)] = hsel^T @ kowT, in N=512 chunks, then
        # per-l PE transposes into kernT [c, (k,l)] with the bias added.
        kernTr = small.tile([128, KL], ADT, tag="kernTr", name="kernTr")
        kernTv = kernTr.rearrange("p (k l) -> p k l", l=L)
        for n0 in range(0, C * L, 512):
            nn = min(512, C * L - n0)
            pk = ps_k.tile([16, 512], F32, tag="pk", name="pkow")
            nc.tensor.matmul(pk[:, 0:nn], hsel[:], kowT[:, n0:n0 + nn],
                             start=True, stop=True)
            kf = stage.tile([16, 512], F32, tag="kfc", name="kfc", bufs=1)
            nc.vector.tensor_copy(kf[:, 0:nn], pk[:, 0:nn])
            for l in range(n0 // 128, (n0 + nn) // 128):
                ptk = ps_small.tile([128, KSEL], F32, tag="pss", name="pss")
                nc.tensor.transpose(ptk, kf[:, 128 * l - n0:128 * l - n0 + 128],
                                    ident[0:16, 0:16])
                nc.vector.tensor_scalar_add(kernTv[:, :, l], ptk,
                                            kobT[:, l:l + 1])

        ab = gn_tail(g32, cc_d[4] if 4 in cc_set else None, mask_gb[:, 8:10], "m")
        apply_gn(mf, ab, HALF, W)

        # ------------- einsum + bf16 output -------------
        for gi, (m0, mm) in enumerate(((0, 128), (128, 128), (256, 128), (384, KL - 384))):
            for g in range(9):
                st = stage.tile([128, 9, W], ODT, tag="stout", name="stout", bufs=2 if out_bf16 else 1)
                rows_g = 0
                for tt in range(3):
                    t = 3 * g + tt
                    if t > 26:
                        continue
                    rows = 3 if t < 26 else 2
                    rows_g += rows
                    ps = ps_conv.tile([128, 3, W], F32, tag="pc", name="pc")
                    nc.tensor.matmul(
                        ps[0:mm, 0:rows, :], kernTr[:, m0:m0 + mm],
                        mf[:, 1 + 3 * t:1 + 3 * t + rows, 1:1 + W],
                        start=True, stop=True)
                    if tt == 1:
                        nc.scalar.copy(out=st[0:mm, 3 * tt:3 * tt + rows, :],
                                       in_=ps[0:mm, 0:rows, :])
                    else:
                        nc.vector.tensor_copy(st[0:mm, 3 * tt:3 * tt + rows, :],
                                              ps[0:mm, 0:rows, :])
                eng = nc.sync if (g + gi) % 2 == 0 else _gp
                eng.dma_start(
                    out=out_d[m0:m0 + mm, 9 * W * g:9 * W * g + rows_g * W],
                    in_=st[0:mm, 0:rows_g, :])

    nc.compile()
    _PROGRAM_CACHE[key] = nc
    return nc


# --------------------------------------------------------------------------
# host glue
# --------------------------------------------------------------------------

def _prep_inputs(inputs, use_f32r=True):
    feats = np.asarray(inputs['feats'], np.float32)
    bboxes = np.asarray(inputs['matched_bboxes'])
    idx = topk_idx(bboxes, KSEL)

    rnd = _round_f32r if use_f32r else (lambda x: np.ascontiguousarray(x, np.float32))

    xx, yy = np.meshgrid(np.linspace(-1, 1, W, dtype=np.float64),
                         np.linspace(-1, 1, H, dtype=np.float64), indexing='xy')
    coord = np.stack([xx, yy]).astype(np.float32)
    Rh = resize_weight_mat(H, S).astype(np.float32)
    Rw = resize_weight_mat(W, S).astype(np.float32)

    def conv_w_prep(w, flip):
        w = np.asarray(w, np.float32)
        if flip:
            w = w[:, :, ::-1, :]
        return np.ascontiguousarray(
            w.transpose(1, 2, 3, 0).reshape(w.shape[1], 9, w.shape[0]))

    def gb_pack(pairs):
        out = np.zeros((128, 2 * len(pairs)), np.float32)
        for i, (g, b) in enumerate(pairs):
            out[:, 2 * i] = np.asarray(g, np.float32)
            out[:, 2 * i + 1] = np.asarray(b, np.float32)
        return out

    mask_gb = gb_pack([(inputs[f'fg{l}'], inputs[f'fb{l}']) for l in range(4)]
                      + [(inputs['fog'], inputs['fob'])])
    k_gb = gb_pack([(inputs[f'kg{l}'], inputs[f'kb{l}']) for l in range(4)])

    kow = np.asarray(inputs['kow'], np.float32).reshape(C * L, C)
    kowT = rnd(np.ascontiguousarray(kow.T))
    kob = np.asarray(inputs['kob'], np.float32)
    kobT = np.ascontiguousarray(kob.reshape(L, C).T)
    fow = np.asarray(inputs['fow'], np.float32).reshape(C, C)
    fowT = rnd(np.ascontiguousarray(fow.T))

    onehot_g = np.zeros((128, 32), np.float32)
    onehot_g[np.arange(128), np.arange(128) // 4] = 1.0
    onehot_b = np.ascontiguousarray(onehot_g.T)

    fw0_full = np.asarray(inputs['fw0'], np.float32)
    kw0_full = np.asarray(inputs['kw0'], np.float32)

    per_orient = {}
    for flip in (False, True):
        wd = {}
        for pre, key in (("f", "fw"), ("k", "kw")):
            w0 = conv_w_prep(inputs[f'{key}0'], flip)
            wd[f"{pre}w0a"] = rnd(w0[0:128])
            wd[f"{pre}w0b"] = rnd(w0[128:256])
            for l in (1, 2, 3):
                wd[f"{pre}w{l}"] = rnd(conv_w_prep(inputs[f'{key}{l}'], flip))
        # coord-contribution maps for the two layer-1 convs
        fw0c = fw0_full[:, 256:258][:, :, ::-1, :] if flip else fw0_full[:, 256:258]
        kw0c = kw0_full[:, 256:258][:, :, ::-1, :] if flip else kw0_full[:, 256:258]
        coord_o = coord[:, ::-1, :] if flip else coord
        t = np.tensordot(Rh, coord_o, axes=(1, 1))
        coordk = np.tensordot(t, Rw, axes=(2, 1)).transpose(1, 0, 2)
        wd["cmapm"] = coord_conv_map(fw0c, coord_o, SLAB)
        wd["cmapk"] = coord_conv_map(kw0c, coordk, S)
        per_orient[flip] = wd

    in_maps = []
    for c in range(8):
        b, s = c // 2, c % 2
        base = feats[b]
        if s == 1:
            base = base[:, ::-1, :]
        img_pad = np.zeros((CIN, ROWS, COLS), np.float32)
        img_pad[:, 1:, 1:1 + W] = base[:, 0:ROWS - 1, :]

        t = np.tensordot(Rh, base, axes=(1, 1))
        kin = np.tensordot(t, Rw, axes=(2, 1)).transpose(1, 0, 2)
        kin_pad = np.zeros((CIN, GRID_P, GRID_P), np.float32)
        kin_pad[:, 1:-1, 1:-1] = kin

        selP = np.zeros((128, NCH * KSEL), np.float32)
        for k in range(KSEL):
            y_, x_ = divmod(int(idx[b, k]), S)
            if s == 1:
                y_ = S - 1 - y_
            p = y_ * S + x_
            selP[p % 128, (p // 128) * KSEL + k] = 1.0

        m = dict(per_orient[s == 1])
        m.update(
            img=rnd(img_pad), kin=rnd(kin_pad),
            fowT=fowT, kowT=kowT, mask_gb=mask_gb, k_gb=k_gb, kobT=kobT,
            onehot_g=onehot_g, onehot_b=onehot_b, selP=selP)
        in_maps.append(m)
    return in_maps


def assemble_output(results):
    out = np.zeros((B, KL, H, W), np.float32)
    for c in range(8):
        b, s = c // 2, c % 2
        pred = np.asarray(results[c]["out"]).astype(np.float32).reshape(KL, HALF, W)
        if s == 0:
            out[b, :, 0:HALF, :] = pred
        else:
            out[b, :, HALF:, :] = pred[:, ::-1, :]
    return out.reshape(B, KSEL, L, H, W)


def kernel(**inputs) -> np.ndarray:
    use_f32r = True
    nc = build_program(use_f32r=use_f32r)
    in_maps = _prep_inputs(inputs, use_f32r=use_f32r)
    res = run_bass_kernel_spmd(nc, in_maps, core_ids=list(range(8)))
    return assemble_output(res.results)


# revision 25
# speedup vs baseline: 1.3520x; 1.0530x over previous
"""Trainium2 Bass kernel for nn_CharacterMaskAttentionHead.

Sharding: 8 cores = 4 images x 2 H-halves (data-parallel over batch, spatial
split within each image). Bottom halves are H-flipped on the host so every core
runs the identical SPMD program on a "top slab" (true image edge at the top,
4-row interior halo at the bottom). GroupNorm statistics are exact: each core
reduces its half, per-pair partial sums are combined with a tiny (32x2 float)
in-kernel AllReduce.

Performance structure (v2):
  - The coord-channel contribution of both layer-1 convs is precomputed on the
    host (coords are fixed ramps) and added on VectorE during the PSUM->SBUF
    copy, removing the 9 extra 2-partition matmuls per tile.
  - bn_stats is batched per conv tile and interleaved with the conv copies, so
    group stats are ready right after the last conv matmul of a layer.
  - Each mask-layer GroupNorm is split into head (reduce + start AllReduce) and
    tail (finish stats -> scale/bias -> apply); the kernel-branch conv of the
    same layer is emitted between head and tail so the PE stays busy during the
    ~11us collective round trip (keeps HAM at full clock).
  - The final fow GN bubble is filled with the top-k gather (PE transpose +
    one-hot matmul) and the kow 1x1 (single weight load, 7 N=512 matmuls).
  - The dynamic-conv einsum output is cast to bf16 on-chip and written with
    grouped DMAs alternating between the Sync and GpSimd queues.
Host: pads/flips/rounds inputs, applies the jax-exact bilinear-antialias
resize, computes top-k, computes the coord maps, reassembles.
"""
import sys
import contextlib

sys.path.insert(0, '/opt/trn_rl_repo')

import numpy as np

import concourse.bacc as bacc
import concourse.bass as bass
import concourse.tile as tile
from concourse import mybir
from concourse.bass_utils import run_bass_kernel_spmd
from concourse.masks import make_identity

F32 = mybir.dt.float32
F32R = mybir.dt.float32r
BF16 = mybir.dt.bfloat16

B, CIN, H, W = 4, 256, 160, 160
C, L, S, KSEL = 128, 25, 40, 16
HALF = 80                 # output rows per core
SLAB = 84                 # conv rows for layer 1 (80 + 4 halo)
ROWS, COLS = 86, 162      # padded slab buffer
GRID_P = S + 2            # padded 40x40 grid (42)
NT = 28                   # mask conv tiles (3 rows x 160)
KL = KSEL * L             # 400
NPOS = S * S              # 1600
NCH = 13                  # ceil(1600/128)
EPS = 1e-5

_PROGRAM_CACHE = {}


def _round_f32r(x):
    """Round fp32 to the f32r grid (12 mantissa bits) so DMA-fed float32r
    tiles hold values the PE will not re-round differently in sim vs HW."""
    b = np.ascontiguousarray(x, np.float32).view(np.uint32)
    b = ((b + 0x800) & np.uint32(0xFFFFF000)).astype(np.uint32)
    return b.view(np.float32)


def resize_weight_mat(in_size, out_size):
    """jax.image.resize bilinear+antialias weight matrix; out = Wmat @ in."""
    inv_scale = in_size / out_size
    kernel_scale = max(inv_scale, 1.0)
    sample_f = (np.arange(out_size) + 0.5) * inv_scale - 0.5
    x = np.abs(sample_f[:, None] - np.arange(in_size)[None, :]) / kernel_scale
    w = np.maximum(0.0, 1.0 - x)
    tot = w.sum(axis=1, keepdims=True)
    w = np.where(np.abs(tot) > 1000 * np.finfo(np.float32).eps, w / tot, 0.0)
    valid = (sample_f >= -0.5) & (sample_f <= in_size - 0.5)
    return np.where(valid[:, None], w, 0.0)


def topk_idx(vals, k):
    """jax.lax.top_k indices: descending value, ties -> lower index first."""
    return np.argsort(-vals, axis=-1, kind='stable')[..., :k]


def coord_conv_map(w0c, cimg, nrows):
    """'SAME' 3x3 conv of the 2-channel coord image with the coord slice of
    the first conv weight; returns [128, nrows, W'] fp32."""
    ch, hh, ww = cimg.shape
    cp = np.zeros((ch, hh + 2, ww + 2), np.float32)
    cp[:, 1:-1, 1:-1] = cimg
    out = np.zeros((w0c.shape[0], hh, ww), np.float32)
    for c in range(ch):
        for dy in range(3):
            for dx in range(3):
                out += (w0c[:, c, dy, dx][:, None, None]
                        * cp[c, dy:dy + hh, dx:dx + ww])
    return np.ascontiguousarray(out[:, 0:nrows])


# --------------------------------------------------------------------------
# device program
# --------------------------------------------------------------------------

def build_program(use_f32r=True, use_cc=True):
    import os as _os
    out_bf16 = _os.environ.get('OUT_BF16', '1') == '1'
    stats_v3 = _os.environ.get('STATS', 'v3') == 'v3'
    cc_set = (set(range(5)) if use_cc is True
              else (set() if use_cc is False else set(use_cc)))
    key = (use_f32r, tuple(sorted(cc_set)), out_bf16, stats_v3)
    if key in _PROGRAM_CACHE:
        return _PROGRAM_CACHE[key]
    ADT = F32R if use_f32r else F32

    nc = bacc.Bacc("TRN2", target_bir_lowering=False, num_devices=8)

    img = nc.dram_tensor("img", (CIN, ROWS, COLS), ADT, kind="ExternalInput")
    kin = nc.dram_tensor("kin", (CIN, GRID_P, GRID_P), ADT, kind="ExternalInput")
    cmapm_d = nc.dram_tensor("cmapm", (128, SLAB, W), F32, kind="ExternalInput")
    cmapk_d = nc.dram_tensor("cmapk", (128, S, S), F32, kind="ExternalInput")
    wdr = {}
    for pre in ("f", "k"):
        wdr[f"{pre}w0a"] = nc.dram_tensor(f"{pre}w0a", (128, 9, 128), ADT, kind="ExternalInput")
        wdr[f"{pre}w0b"] = nc.dram_tensor(f"{pre}w0b", (128, 9, 128), ADT, kind="ExternalInput")
        for l in (1, 2, 3):
            wdr[f"{pre}w{l}"] = nc.dram_tensor(f"{pre}w{l}", (128, 9, 128), ADT, kind="ExternalInput")
    fowT_d = nc.dram_tensor("fowT", (128, 128), ADT, kind="ExternalInput")
    kowT_d = nc.dram_tensor("kowT", (128, C * L), ADT, kind="ExternalInput")
    mask_gb_d = nc.dram_tensor("mask_gb", (128, 10), F32, kind="ExternalInput")
    k_gb_d = nc.dram_tensor("k_gb", (128, 8), F32, kind="ExternalInput")
    kobT_d = nc.dram_tensor("kobT", (128, L), F32, kind="ExternalInput")
    onehot_g_d = nc.dram_tensor("onehot_g", (128, 32), F32, kind="ExternalInput")
    onehot_b_d = nc.dram_tensor("onehot_b", (32, 128), F32, kind="ExternalInput")
    selP_d = nc.dram_tensor("selP", (128, NCH * KSEL), F32, kind="ExternalInput")
    ODT = BF16 if out_bf16 else F32
    out_d = nc.dram_tensor("out", (KL, HALF * W), ODT, kind="ExternalOutput")
    cc_d = [(nc.dram_tensor(f"ccin{l}", (32, 2), F32),
             nc.dram_tensor(f"ccout{l}", (32, 2), F32)) for l in range(6)]

    with tile.TileContext(nc) as tc, contextlib.ExitStack() as ctx:
        consts = ctx.enter_context(tc.tile_pool(name="consts", bufs=1))
        acts = ctx.enter_context(tc.tile_pool(name="acts", bufs=1))
        wpool = ctx.enter_context(tc.tile_pool(name="wpool", bufs=2))
        small = ctx.enter_context(tc.tile_pool(name="small", bufs=1))
        stage = ctx.enter_context(tc.tile_pool(name="stage", bufs=2))
        kpool = ctx.enter_context(tc.tile_pool(name="kpool", bufs=1))
        ps_conv = ctx.enter_context(tc.tile_pool(name="ps_conv", bufs=4, space="PSUM"))
        ps_k = ctx.enter_context(tc.tile_pool(name="ps_k", bufs=2, space="PSUM"))
        ps_small = ctx.enter_context(tc.tile_pool(name="ps_small", bufs=2, space="PSUM"))

        import os
        _dma = {'sync': nc.sync, 'scalar': nc.scalar, 'gpsimd': nc.gpsimd}[os.environ.get('DMA_ENG', 'mixed')] if os.environ.get('DMA_ENG', 'mixed') != 'mixed' else None
        _sc = _dma or nc.scalar
        _gp = _dma or nc.gpsimd
        # ------------- constants -------------
        def load_const(dram, shape, dt=F32, eng=None):
            t = consts.tile(shape, dt, tag=dram.name, name=dram.name)
            (eng or nc.scalar).dma_start(out=t, in_=dram[tuple(slice(None) for _ in shape)])
            return t

        onehot_g = load_const(onehot_g_d, [128, 32])
        onehot_b = load_const(onehot_b_d, [32, 128])
        mask_gb = load_const(mask_gb_d, [128, 10])
        k_gb = load_const(k_gb_d, [128, 8])
        kobT = load_const(kobT_d, [128, L])
        selP = load_const(selP_d, [128, NCH * KSEL])
        fowT = load_const(fowT_d, [128, 128], ADT)
        cmapk = load_const(cmapk_d, [128, S, S], F32, eng=nc.gpsimd)
        eps32 = consts.tile([32, 1], F32, tag="eps32", name="eps32")
        nc.vector.memset(eps32, EPS)
        ident = consts.tile([128, 128], F32, tag="ident", name="ident")
        make_identity(nc, ident)

        # activation slabs (ping-pong); pads zeroed via tensor_copy from an
        # F32 zero tile (memset cannot produce f32r, tensor_copy can)
        zsrc = consts.tile([128, COLS], F32, tag="zsrc", name="zsrc")
        nc.vector.memset(zsrc, 0.0)
        zcol = zsrc[:, 0:ROWS].rearrange("p (a b) -> p a b", b=1)

        # conv weights first on the sync queue so layer 1 can start ASAP
        def load_w(nm, tag):
            dr = wdr[nm]
            t = wpool.tile([dr.shape[0], 9, 128], ADT, tag=tag, name=nm)
            nc.sync.dma_start(out=t, in_=dr[:, :, :])
            return t

        fw0 = [load_w("fw0a", "fw"), load_w("fw0b", "fw")]
        kw0 = [load_w("kw0a", "kw"), load_w("kw0b", "kw")]

        l1_stages = {}

        def l1_src(t, ci):
            if (t, ci) not in l1_stages:
                st = stage.tile([128, 5, COLS], ADT, tag=f"st{ci}", name=f"st{ci}")
                nc.sync.dma_start(out=st, in_=img[128 * ci:128 * ci + 128,
                                                  3 * t:3 * t + 5, :])
                l1_stages[(t, ci)] = st
            return l1_stages[(t, ci)]

        cm_stages = {}

        def l1_cmap(t):
            if t not in cm_stages:
                cm = stage.tile([128, 3, W], F32, tag="cm", name="cm")
                nc.sync.dma_start(out=cm, in_=cmapm_d[:, 3 * t:3 * t + 3, :])
                cm_stages[t] = cm
            return cm_stages[t]

        for _t in (0, 1):
            l1_src(_t, 0), l1_src(_t, 1), l1_cmap(_t)

        X = [acts.tile([128, ROWS, COLS], ADT, tag="X0", name="X0"),
             acts.tile([128, ROWS, COLS], ADT, tag="X1", name="X1")]
        for x in X:
            nc.vector.tensor_copy(x[:, 0, :], zsrc[:, :])
            nc.vector.tensor_copy(x[:, ROWS - 1, :], zsrc[:, :])
            nc.vector.tensor_copy(x[:, :, 0:1], zcol)
            nc.vector.tensor_copy(x[:, :, COLS - 1:COLS], zcol)
        KB = [kpool.tile([128, GRID_P, GRID_P], ADT, tag="KB0", name="KB0"),
              kpool.tile([128, GRID_P, GRID_P], ADT, tag="KB1", name="KB1")]
        zcolk = zsrc[:, 0:GRID_P].rearrange("p (a b) -> p a b", b=1)
        for x in KB:
            nc.vector.tensor_copy(x[:, 0, :], zsrc[:, 0:GRID_P])
            nc.vector.tensor_copy(x[:, GRID_P - 1, :], zsrc[:, 0:GRID_P])
            nc.vector.tensor_copy(x[:, :, 0:1], zcolk)
            nc.vector.tensor_copy(x[:, :, GRID_P - 1:GRID_P], zcolk)

        # kin chunks (tags shared with later k-branch tiles to save SBUF)
        kinA = kpool.tile([128, GRID_P, GRID_P], ADT, tag="sh_kow", name="sh_kow")
        kinB = kpool.tile([128, GRID_P, GRID_P], ADT, tag="sh_hidT", name="sh_hidT")
        _gp.dma_start(out=kinA, in_=kin[0:128, :, :])
        _gp.dma_start(out=kinB, in_=kin[128:256, :, :])

        # warm the collectives firmware path so the first real AllReduce
        # does not pay the ~11us first-call start delay (runs under L1 conv)
        if cc_set and os.environ.get('NO_WARM_CC', '') != '1':
            _gp.dma_start(out=cc_d[5][0][:, :], in_=zsrc[0:32, 0:2])
            nc.gpsimd.collective_compute(
                "AllReduce", mybir.AluOpType.add,
                replica_groups=[[0, 1], [2, 3], [4, 5], [6, 7]],
                ins=[cc_d[5][0][:, :]], outs=[cc_d[5][1][:, :]])

        # ------------- GroupNorm helpers -------------
        # per-conv-tile partial sums / sums-of-squares (fused into the
        # PSUM->SBUF copy via tensor_tensor_reduce)
        zeros3 = consts.tile([128, 3, W], F32, tag="zeros3", name="zeros3")
        nc.vector.memset(zeros3, 0.0)
        sqd = small.tile([128, 3, W], F32, tag="sqd", name="sqd")
        ksqd = small.tile([128, 10, S], F32, tag="ksqd", name="ksqd")
        kzeros = consts.tile([128, 10, S], F32, tag="kzeros", name="kzeros")
        nc.vector.memset(kzeros, 0.0)
        statsm = small.tile([128, HALF, 6], F32, tag="bnstm", name="bnstm")
        statsk = small.tile([128, S, 6], F32, tag="bnstk", name="bnstk")
        msum = small.tile([128, NT], F32, tag="msum", name="msum")
        msqs = small.tile([128, NT], F32, tag="msqs", name="msqs")
        ksum = small.tile([128, 4], F32, tag="ksum", name="ksum")
        ksqs = small.tile([128, 4], F32, tag="ksqs", name="ksqs")

        def gn_head(sums, sqs, nvalid, divisor, cc, fam):
            """reduce tile partials -> group-reduce -> (start AllReduce).
            Returns the local [32,2] (mean|E[x^2]) tile, pre-divided."""
            sq = small.tile([128, 2], F32, tag=f"sq{fam}", name=f"sq{fam}")
            if stats_v3:
                nc.vector.tensor_reduce(out=sq[:, 0:1], in_=sums[:, 0:nvalid],
                                        axis=mybir.AxisListType.X,
                                        op=mybir.AluOpType.add)
                nc.vector.tensor_reduce(out=sq[:, 1:2], in_=sqs[:, 0:nvalid],
                                        axis=mybir.AxisListType.X,
                                        op=mybir.AluOpType.add)
            else:
                stats, nrows, npx = sums
                mv = small.tile([128, 2], F32, tag=f"mv{fam}", name=f"mv{fam}")
                nc.vector.bn_aggr(out=mv[:], in_=stats[:, 0:nrows, :]
                                  .rearrange("p a b -> p (a b)"))
                nc.vector.tensor_mul(sq[:, 1:2], mv[:, 0:1], mv[:, 0:1])
                nc.vector.tensor_add(sq[:, 1:2], sq[:, 1:2], mv[:, 1:2])
                nc.vector.tensor_copy(sq[:, 0:1], mv[:, 0:1])
                nc.vector.tensor_scalar_mul(sq[:], sq[:], float(npx))
            pg = ps_small.tile([32, 2], F32, tag="pss", name="pss")
            nc.tensor.matmul(pg, onehot_g[:], sq[:], start=True, stop=True)
            g32 = small.tile([32, 2], F32, tag=f"g32{fam}", name=f"g32{fam}")
            nc.vector.tensor_scalar_mul(g32[:], pg, 1.0 / divisor)
            if cc is not None:
                ccin_d, ccout_d = cc
                _sc.dma_start(out=ccin_d[:, :], in_=g32[:])
                nc.gpsimd.collective_compute(
                    "AllReduce", mybir.AluOpType.add,
                    replica_groups=[[0, 1], [2, 3], [4, 5], [6, 7]],
                    ins=[ccin_d[:, :]], outs=[ccout_d[:, :]])
            return g32

        def gn_tail(g32, cc, gb_ap, fam):
            """finish group stats -> per-channel scale(a)|bias(b) [128,2]."""
            if cc is not None:
                g32 = small.tile([32, 2], F32, tag=f"g32r{fam}", name=f"g32r{fam}")
                _sc.dma_start(out=g32[:], in_=cc[1][:, :])
            msq = small.tile([32, 1], F32, tag=f"msq{fam}", name=f"msq{fam}")
            nc.vector.tensor_mul(msq[:], g32[:, 0:1], g32[:, 0:1])
            nc.vector.tensor_sub(g32[:, 1:2], g32[:, 1:2], msq[:])
            nc.scalar.activation(out=g32[:, 1:2], in_=g32[:, 1:2],
                                 func=mybir.ActivationFunctionType.Sqrt,
                                 bias=eps32[:], scale=1.0)
            nc.vector.reciprocal(g32[:, 1:2], g32[:, 1:2])
            pb = ps_small.tile([128, 2], F32, tag="pss", name="pss")
            nc.tensor.matmul(pb, onehot_b[:], g32[:], start=True, stop=True)
            mr = small.tile([128, 2], F32, tag=f"mr{fam}", name=f"mr{fam}")
            nc.vector.tensor_copy(mr[:], pb)
            ab = small.tile([128, 2], F32, tag=f"ab{fam}", name=f"ab{fam}")
            nc.vector.tensor_mul(ab[:, 0:1], gb_ap[:, 0:1], mr[:, 1:2])
            nc.vector.tensor_mul(ab[:, 1:2], mr[:, 0:1], ab[:, 0:1])
            nc.vector.tensor_sub(ab[:, 1:2], gb_ap[:, 1:2], ab[:, 1:2])
            return ab

        def apply_gn(dst, ab, nrows, ncols, first=6, chunk=13):
            r = 0
            while r < nrows:
                take = min(first if r == 0 else chunk, nrows - r)
                nc.scalar.activation(
                    out=dst[:, 1 + r:1 + r + take, 1:1 + ncols],
                    in_=dst[:, 1 + r:1 + r + take, 1:1 + ncols].bitcast(F32),
                    func=mybir.ActivationFunctionType.Relu,
                    scale=ab[:, 0:1], bias=ab[:, 1:2])
                r += take

        # ------------- conv emitters (stats fused into the copy) -------------
        def copy_stats(dst_ap, ps, in1, rows_v, rows, slot_s, slot_q,
                       dump=None):
            """dst = ps (+ in1), sum via VectorE tensor_reduce and
            sum-of-squares via a ScalarE Square activation with accum_out."""
            if in1 is None:
                nc.vector.tensor_copy(dst_ap, ps)
            else:
                nc.vector.tensor_add(dst_ap, ps, in1)
            if rows_v == 0:
                return
            d32 = dst_ap[:, 0:rows_v, :].bitcast(F32)
            nc.vector.tensor_reduce(out=slot_s, in_=d32,
                                    axis=mybir.AxisListType.XY,
                                    op=mybir.AluOpType.add)
            dmp = dump if dump is not None else sqd
            nc.scalar.activation(out=dmp[:, 0:rows_v, 0:dst_ap.shape[2]],
                                 in_=d32,
                                 func=mybir.ActivationFunctionType.Square,
                                 accum_out=slot_q)

        def copy_stats_bn(dst_ap, ps, in1, rows_v, rows, stats, srow):
            nc.vector.tensor_add(dst_ap, ps, in1)
            for r in range(rows_v):
                nc.vector.bn_stats(out=stats[:, srow + r, :],
                                   in_=dst_ap[:, r, :].bitcast(F32))

        def mask_conv_layer(src_fn, wts, dst, ntiles=NT, cmap_fn=None):
            n_chunks = len(wts)
            for t in range(ntiles):
                ps = ps_conv.tile([128, 3, W], F32, tag="pc", name="pc")
                first = True
                for ci in range(n_chunks):
                    rhs_tile = src_fn(t, ci)
                    for tap in range(9):
                        dy, dx = divmod(tap, 3)
                        nc.tensor.matmul(
                            ps, wts[ci][:, tap, :],
                            rhs_tile[:, dy:dy + 3, dx:dx + W],
                            start=first,
                            stop=(ci == n_chunks - 1 and tap == 8))
                        first = False
                in1 = cmap_fn(t) if cmap_fn is not None else None
                rows_v = 3 if t <= 25 else (2 if t == 26 else 0)
                if stats_v3:
                    copy_stats(dst[:, 1 + 3 * t:4 + 3 * t, 1:1 + W], ps, in1,
                               rows_v, 3, msum[:, t:t + 1], msqs[:, t:t + 1])
                else:
                    copy_stats_bn(dst[:, 1 + 3 * t:4 + 3 * t, 1:1 + W], ps,
                                  in1 if in1 is not None else zeros3,
                                  rows_v, 3, statsm, 3 * t)

        def k_conv_layer(srcs, wts, dst, cmap=None):
            n_chunks = len(wts)
            for t in range(4):
                ps = ps_k.tile([128, 10, S], F32, tag="pk", name="pk")
                first = True
                for ci in range(n_chunks):
                    for tap in range(9):
                        dy, dx = divmod(tap, 3)
                        nc.tensor.matmul(
                            ps, wts[ci][:, tap, :],
                            srcs[ci][:, 10 * t + dy:10 * t + dy + 10, dx:dx + S],
                            start=first,
                            stop=(ci == n_chunks - 1 and tap == 8))
                        first = False
                dst_ap = dst[:, 1 + 10 * t:11 + 10 * t, 1:1 + S]
                kin1 = (cmap[:, 10 * t:10 * t + 10, :] if cmap is not None
                        else None)
                if stats_v3:
                    copy_stats(dst_ap, ps, kin1, 10, 10, ksum[:, t:t + 1],
                               ksqs[:, t:t + 1], dump=ksqd)
                else:
                    copy_stats_bn(dst_ap, ps,
                                  kin1 if kin1 is not None else kzeros,
                                  10, 10, statsk, 10 * t)

        def k_gn_apply(buf, l):
            g32k = gn_head(ksum if stats_v3 else (statsk, S, S * S),
                           ksqs, 4, 4.0 * S * S, None, "k")
            abk = gn_tail(g32k, None, k_gb[:, 2 * l:2 * l + 2], "k")
            apply_gn(buf, abk, S, S, first=20, chunk=20)

        # ------------- mask layer 1 (streamed from DRAM) + k layer 1 -------------
        mask_conv_layer(l1_src, fw0, X[0], cmap_fn=l1_cmap)
        g32 = gn_head(msum if stats_v3 else (statsm, HALF, HALF * W),
                      msqs, 27, 8.0 * HALF * W,
                      cc_d[0] if 0 in cc_set else None, "m")
        # ---- bubble filler: k-tower layer 1 ----
        k_conv_layer([kinA, kinB], kw0, KB[0], cmap=cmapk)
        k_gn_apply(KB[0], 0)
        ab = gn_tail(g32, cc_d[0] if 0 in cc_set else None, mask_gb[:, 0:2], "m")
        apply_gn(X[0], ab, SLAB, W)

        # ------------- layers 2..4 -------------
        cur = 0
        kcur = 0
        for l in (1, 2, 3):
            fwl = [load_w(f"fw{l}", "fw")]
            kwl = [load_w(f"kw{l}", "kw")]

            def src(t, ci, _cur=cur):
                return X[_cur][:, 3 * t:3 * t + 5, :]

            ntiles = 27 if l == 3 else NT
            mask_conv_layer(src, fwl, X[1 - cur], ntiles=ntiles)
            g32 = gn_head(msum if stats_v3 else (statsm, HALF, HALF * W),
                          msqs, 27, 8.0 * HALF * W,
                          cc_d[l] if l in cc_set else None, "m")
            # ---- bubble filler: k-tower layer l ----
            k_conv_layer([KB[kcur]], kwl, KB[1 - kcur])
            if l == 1:
                # kow weights: load into kinA's slot (dead after k layer 1)
                kowT = kpool.tile([128, C * L], ADT, tag="sh_kow", name="kowT")
                nc.sync.dma_start(out=kowT, in_=kowT_d[:, :])
            k_gn_apply(KB[1 - kcur], l)
            if l == 3:
                # ---- bubble filler: top-k gather off the finished k tower ----
                khid = KB[1 - kcur]
                hidc = kpool.tile([128, S, S], F32, tag="KB0", name="hidc")
                nc.vector.tensor_copy(hidc[:], khid[:, 1:1 + S, 1:1 + S].bitcast(F32))
                hidcf = hidc.rearrange("p a b -> p (a b)")
                hidT = kpool.tile([128, NCH, 128], F32, tag="sh_hidT", name="hidT")
                for j in range(NCH):
                    npos = min(128, NPOS - 128 * j)
                    pt = ps_small.tile([128, 128], F32, tag="pss", name="pss")
                    nc.tensor.transpose(pt[0:npos, :],
                                        hidcf[:, 128 * j:128 * j + npos], ident)
                    nc.vector.tensor_copy(hidT[0:npos, j, :], pt[0:npos, :])
                psel = ps_small.tile([128, KSEL], F32, tag="pss", name="pss")
                selPv = selP.rearrange("p (j k) -> p j k", j=NCH)
                for j in range(NCH):
                    npos = min(128, NPOS - 128 * j)
                    nc.tensor.matmul(psel, hidT[0:npos, j, :],
                                     selPv[0:npos, j, :],
                                     start=(j == 0), stop=(j == NCH - 1))
                hsel = small.tile([128, KSEL], ADT, tag="hsel", name="hsel")
                nc.vector.tensor_copy(hsel[:], psel)
            ab = gn_tail(g32, cc_d[l] if l in cc_set else None,
                         mask_gb[:, 2 * l:2 * l + 2], "m")
            apply_gn(X[1 - cur], ab, SLAB if l < 3 else 81, W)
            cur, kcur = 1 - cur, 1 - kcur

        # ------------- fow + GN (+ gather/kow in the collective bubble) -----
        mf = X[1 - cur]
        for t in range(27):
            ps = ps_conv.tile([128, 3, W], F32, tag="pc", name="pc")
            nc.tensor.matmul(ps, fowT[:],
                             X[cur][:, 1 + 3 * t:4 + 3 * t, 1:1 + W],
                             start=True, stop=True)
            rows_v = 3 if t <= 25 else 2
            if stats_v3:
                copy_stats(mf[:, 1 + 3 * t:4 + 3 * t, 1:1 + W], ps, None,
                           rows_v, 3, msum[:, t:t + 1], msqs[:, t:t + 1])
            else:
                copy_stats_bn(mf[:, 1 + 3 * t:4 + 3 * t, 1:1 + W], ps,
                              zeros3, rows_v, 3, statsm, 3 * t)
        g32 = gn_head(msum if stats_v3 else (statsm, HALF, HALF * W),
                      msqs, 27, 8.0 * HALF * W,
                      cc_d[4] if 4 in cc_set else None, "m")

        # ---- bubble filler: kow ----
        # kow: kern^T[(k), (l,c)] = hsel^T @ kowT, in N=512 chunks, then
        # per-l PE transposes into kernT [c, (k,l)] with the bias added.
        kernTr = small.tile([128, KL], ADT, tag="kernTr", name="kernTr")
        kernTv = kernTr.rearrange("p (k l) -> p k l", l=L)
        for n0 in range(0, C * L, 512):
            nn = min(512, C * L - n0)
            pk = ps_k.tile([16, 512], F32, tag="pk", name="pkow")
            nc.tensor.matmul(pk[:, 0:nn], hsel[:], kowT[:, n0:n0 + nn],
                             start=True, stop=True)
            kf = stage.tile([16, 512], F32, tag="kfc", name="kfc", bufs=1)
            nc.vector.tensor_copy(kf[:, 0:nn], pk[:, 0:nn])
            for l in range(n0 // 128, (n0 + nn) // 128):
                ptk = ps_small.tile([128, KSEL], F32, tag="pss", name="pss")
                nc.tensor.transpose(ptk, kf[:, 128 * l - n0:128 * l - n0 + 128],
                                    ident[0:16, 0:16])
                nc.vector.tensor_scalar_add(kernTv[:, :, l], ptk,
                                            kobT[:, l:l + 1])

        ab = gn_tail(g32, cc_d[4] if 4 in cc_set else None, mask_gb[:, 8:10], "m")
        apply_gn(mf, ab, HALF, W)

        # ------------- einsum + bf16 output -------------
        for gi, (m0, mm) in enumerate(((0, 128), (128, 128), (256, 128), (384, KL - 384))):
            for g in range(9):
                st = stage.tile([128, 9, W], ODT, tag="stout", name="stout", bufs=2 if out_bf16 else 1)
                rows_g = 0
                for tt in range(3):
                    t = 3 * g + tt
                    if t > 26:
                        continue
                    rows = 3 if t < 26 else 2
                    rows_g += rows
                    ps = ps_conv.tile([128, 3, W], F32, tag="pc", name="pc")
                    nc.tensor.matmul(
                        ps[0:mm, 0:rows, :], kernTr[:, m0:m0 + mm],
                        mf[:, 1 + 3 * t:1 + 3 * t + rows, 1:1 + W],
                        start=True, stop=True)
                    if tt == 1:
                        nc.scalar.copy(out=st[0:mm, 3 * tt:3 * tt + rows, :],
                                       in_=ps[0:mm, 0:rows, :])
                    else:
                        nc.vector.tensor_copy(st[0:mm, 3 * tt:3 * tt + rows, :],
                                              ps[0:mm, 0:rows, :])
                eng = (nc.sync, nc.scalar, nc.gpsimd)[(9 * gi + g) % 3]
                eng.dma_start(
                    out=out_d[m0:m0 + mm, 9 * W * g:9 * W * g + rows_g * W],
                    in_=st[0:mm, 0:rows_g, :])

    nc.compile()
    _PROGRAM_CACHE[key] = nc
    return nc


# --------------------------------------------------------------------------
# host glue
# --------------------------------------------------------------------------

def _prep_inputs(inputs, use_f32r=True):
    feats = np.asarray(inputs['feats'], np.float32)
    bboxes = np.asarray(inputs['matched_bboxes'])
    idx = topk_idx(bboxes, KSEL)

    rnd = _round_f32r if use_f32r else (lambda x: np.ascontiguousarray(x, np.float32))

    xx, yy = np.meshgrid(np.linspace(-1, 1, W, dtype=np.float64),
                         np.linspace(-1, 1, H, dtype=np.float64), indexing='xy')
    coord = np.stack([xx, yy]).astype(np.float32)
    Rh = resize_weight_mat(H, S).astype(np.float32)
    Rw = resize_weight_mat(W, S).astype(np.float32)

    def conv_w_prep(w, flip):
        w = np.asarray(w, np.float32)
        if flip:
            w = w[:, :, ::-1, :]
        return np.ascontiguousarray(
            w.transpose(1, 2, 3, 0).reshape(w.shape[1], 9, w.shape[0]))

    def gb_pack(pairs):
        out = np.zeros((128, 2 * len(pairs)), np.float32)
        for i, (g, b) in enumerate(pairs):
            out[:, 2 * i] = np.asarray(g, np.float32)
            out[:, 2 * i + 1] = np.asarray(b, np.float32)
        return out

    mask_gb = gb_pack([(inputs[f'fg{l}'], inputs[f'fb{l}']) for l in range(4)]
                      + [(inputs['fog'], inputs['fob'])])
    k_gb = gb_pack([(inputs[f'kg{l}'], inputs[f'kb{l}']) for l in range(4)])

    kow = np.asarray(inputs['kow'], np.float32).reshape(C * L, C)
    kowT = rnd(np.ascontiguousarray(kow.T))
    kob = np.asarray(inputs['kob'], np.float32)
    kobT = np.ascontiguousarray(kob.reshape(L, C).T)
    fow = np.asarray(inputs['fow'], np.float32).reshape(C, C)
    fowT = rnd(np.ascontiguousarray(fow.T))

    onehot_g = np.zeros((128, 32), np.float32)
    onehot_g[np.arange(128), np.arange(128) // 4] = 1.0
    onehot_b = np.ascontiguousarray(onehot_g.T)

    fw0_full = np.asarray(inputs['fw0'], np.float32)
    kw0_full = np.asarray(inputs['kw0'], np.float32)

    per_orient = {}
    for flip in (False, True):
        wd = {}
        for pre, key in (("f", "fw"), ("k", "kw")):
            w0 = conv_w_prep(inputs[f'{key}0'], flip)
            wd[f"{pre}w0a"] = rnd(w0[0:128])
            wd[f"{pre}w0b"] = rnd(w0[128:256])
            for l in (1, 2, 3):
                wd[f"{pre}w{l}"] = rnd(conv_w_prep(inputs[f'{key}{l}'], flip))
        # coord-contribution maps for the two layer-1 convs
        fw0c = fw0_full[:, 256:258][:, :, ::-1, :] if flip else fw0_full[:, 256:258]
        kw0c = kw0_full[:, 256:258][:, :, ::-1, :] if flip else kw0_full[:, 256:258]
        coord_o = coord[:, ::-1, :] if flip else coord
        t = np.tensordot(Rh, coord_o, axes=(1, 1))
        coordk = np.tensordot(t, Rw, axes=(2, 1)).transpose(1, 0, 2)
        wd["cmapm"] = coord_conv_map(fw0c, coord_o, SLAB)
        wd["cmapk"] = coord_conv_map(kw0c, coordk, S)
        per_orient[flip] = wd

    in_maps = []
    for c in range(8):
        b, s = c // 2, c % 2
        base = feats[b]
        if s == 1:
            base = base[:, ::-1, :]
        img_pad = np.zeros((CIN, ROWS, COLS), np.float32)
        img_pad[:, 1:, 1:1 + W] = base[:, 0:ROWS - 1, :]

        t = np.tensordot(Rh, base, axes=(1, 1))
        kin = np.tensordot(t, Rw, axes=(2, 1)).transpose(1, 0, 2)
        kin_pad = np.zeros((CIN, GRID_P, GRID_P), np.float32)
        kin_pad[:, 1:-1, 1:-1] = kin

        selP = np.zeros((128, NCH * KSEL), np.float32)
        for k in range(KSEL):
            y_, x_ = divmod(int(idx[b, k]), S)
            if s == 1:
                y_ = S - 1 - y_
            p = y_ * S + x_
            selP[p % 128, (p // 128) * KSEL + k] = 1.0

        m = dict(per_orient[s == 1])
        m.update(
            img=rnd(img_pad), kin=rnd(kin_pad),
            fowT=fowT, kowT=kowT, mask_gb=mask_gb, k_gb=k_gb, kobT=kobT,
            onehot_g=onehot_g, onehot_b=onehot_b, selP=selP)
        in_maps.append(m)
    return in_maps


def assemble_output(results):
    out = np.zeros((B, KL, H, W), np.float32)
    for c in range(8):
        b, s = c // 2, c % 2
        pred = np.asarray(results[c]["out"]).astype(np.float32).reshape(KL, HALF, W)
        if s == 0:
            out[b, :, 0:HALF, :] = pred
        else:
            out[b, :, HALF:, :] = pred[:, ::-1, :]
    return out.reshape(B, KSEL, L, H, W)


def kernel(**inputs) -> np.ndarray:
    use_f32r = True
    nc = build_program(use_f32r=use_f32r)
    in_maps = _prep_inputs(inputs, use_f32r=use_f32r)
    res = run_bass_kernel_spmd(nc, in_maps, core_ids=list(range(8)))
    return assemble_output(res.results)
